# revision 8
# baseline (speedup 1.0000x reference)
"""Decagon GNN (2-layer multi-relation GCN) on 8 TRN2 NeuronCores.

Destination-node sharding across 8 cores. Edges routed by destination
partition, sorted by destination row, packed into 128-edge tiles. Per tile:
indirect DMA gather (dma_gather) of projected source rows + a TensorE one-hot
matmul performing the val-weighted segment-sum in PSUM.

dma_gather transfers must be multiples of 256B, so projection slices are
packed in pairs: each source array holds two 64-col bf16 slices (256B rows)
and each fragment's matmul consumes its 64-col half. This keeps every
gathered row at the 256B minimum (the 4-queue SWDGE descriptor pipeline is
the bottleneck; its time scales with bytes per descriptor).

Launch A: projections X = feat @ W1 (project-first), layer-1 scatter -> relu
h shards (row-major bf16, padded to 128 cols).
Launch B: host all-gathers h shards, layer-2 scatter -> transposed segment
sums S.T, dense W2 matmuls -> z.T shards. Host assembles [70000, 32] f32.
"""
import sys
import numpy as np

sys.path.insert(0, "/opt/trn_rl_repo")

import ml_dtypes
import concourse.bacc as bacc
import concourse.mybir as mybir
import concourse.tile as tile
from concourse.bass_utils import run_bass_kernel_spmd

BF16 = ml_dtypes.bfloat16
P = 128
NCORES = 8
N0, N1, F, H1, H2, K11 = 50000, 20000, 128, 64, 32, 3
D0, D1 = N0 // NCORES, N1 // NCORES          # 6250, 2500
GRP = 64
G0 = -(-D0 // GRP)                            # 98
G1 = -(-D1 // GRP)                            # 40
PAD0, PAD1 = G0 * GRP, G1 * GRP               # 6272, 2560
BANK = 32768
NB = 4                                        # groups per batch
NIPC = 4096                                   # idxs per dma_gather call
NQ = 4                                        # SWDGE queues
CH = 16                                       # 128-row chunks per proj stage
FP32 = mybir.dt.float32
BF = mybir.dt.bfloat16
I16 = mybir.dt.int16

# padded projection-array row counts (full-chunk stores)
NCH0 = -(-N0 // P)                            # 391
NCH1 = -(-N1 // P)                            # 157
X0ROWS = NCH0 * P                             # 50048
X1ROWS = NCH1 * P                             # 20096
X0B_ROWS = X0ROWS - BANK                      # 17280

HW_TIMES = {}


# ---------------------------------------------------------------- host prep

def _prep(e):
    """Route edges, build the static schedule and per-core idx/onehot images.

    Fragment src ids: 0 = bank A of the N0-source array, 2 = bank B,
    1 = the N1-source array. sl = projection slice of the fragment.
    """
    frag_meta = [[], []]
    per_core = [[[], []] for _ in range(NCORES)]

    def add_frag(dt, rows, cols, vals, dper, src, sl, boff=0, mask_extra=None):
        frag_meta[dt].append((src, sl))
        for c in range(NCORES):
            m = (rows // dper) == c
            if mask_extra is not None:
                m = m & mask_extra
            d = (rows[m] - c * dper).astype(np.int64)
            cc = (cols[m] - boff).astype(np.int64)
            vv = vals[m]
            o = np.argsort(d, kind='stable')
            d, cc, vv = d[o], cc[o], vv[o]
            ng = G0 if dt == 0 else G1
            per_core[c][dt].append(dict(
                d=d, col=cc, val=vv,
                cnt=np.bincount(d // GRP, minlength=ng)))

    ba00 = np.asarray(e['e00_col'] < BANK)
    add_frag(0, e['e00_row'], e['e00_col'], e['e00_val'], D0, 0, 0, 0, ba00)
    add_frag(0, e['e00_row'], e['e00_col'], e['e00_val'], D0, 2, 0, BANK, ~ba00)
    add_frag(0, e['e01_row'], e['e01_col'], e['e01_val'], D0, 1, 0)
    ba10 = np.asarray(e['e10_col'] < BANK)
    add_frag(1, e['e10_row'], e['e10_col'], e['e10_val'], D1, 0, 1, 0, ba10)
    add_frag(1, e['e10_row'], e['e10_col'], e['e10_val'], D1, 2, 1, BANK, ~ba10)
    for k in range(K11):
        add_frag(1, e['e11_row'][k], e['e11_col'][k], e['e11_val'][k],
                 D1, 1, k + 1)

    T = []
    for dt in range(2):
        ng = G0 if dt == 0 else G1
        nf = len(frag_meta[dt])
        Tdt = np.zeros((nf, ng), dtype=np.int64)
        for f in range(nf):
            for c in range(NCORES):
                Tdt[f] = np.maximum(
                    Tdt[f], (per_core[c][dt][f]['cnt'] + 127) // 128)
        T.append(Tdt)

    calls = []        # (src_id, sl, icol0, nipc, ti0, ntl)
    tiles = []        # (call_i, tl, l1_slice, dt, g, src)
    batches = []      # (dt, [groups], call0, ncalls, tile0, ntiles)
    frag_tiles = [[[] for _ in range(T[dt].shape[0])] for dt in range(2)]
    icol = 0
    for dt in range(2):
        ng = G0 if dt == 0 else G1
        for b0 in range(0, ng, NB):
            gs = list(range(b0, min(b0 + NB, ng)))
            bc0, bt0 = len(calls), len(tiles)
            for f in range(T[dt].shape[0]):
                src, sl = frag_meta[dt][f]
                tl_list = [(g, t) for g in gs for t in range(T[dt][f][g])]
                pos = 0
                while pos < len(tl_list):
                    ct = tl_list[pos:pos + NIPC // 128]
                    ci = len(calls)
                    calls.append((src, sl, icol, len(ct) * 128,
                                  len(tiles), len(ct)))
                    for tl, (g, t) in enumerate(ct):
                        frag_tiles[dt][f].append(len(tiles))
                        tiles.append((ci, tl, sl, dt, g, src))
                    icol += len(ct) * 8
                    pos += len(ct)
            batches.append((dt, gs, bc0, len(calls) - bc0, bt0,
                            len(tiles) - bt0))
    C_total, n_tiles = icol, len(tiles)

    idx_imgs, oh_imgs, slot_dbg = [], [], []
    for c in range(NCORES):
        gidx = np.zeros(n_tiles * 128, dtype=np.int64)
        glrow = np.zeros(n_tiles * 128, dtype=np.int64)
        gval = np.zeros(n_tiles * 128, dtype=np.float32)
        for dt in range(2):
            for f in range(T[dt].shape[0]):
                fr = per_core[c][dt][f]
                if len(fr['d']) == 0:
                    continue
                Tcum = np.concatenate([[0], np.cumsum(T[dt][f])])
                gstart = np.concatenate([[0], np.cumsum(fr['cnt'])])
                gid = fr['d'] // GRP
                rank = np.arange(len(fr['d'])) - gstart[gid]
                j = Tcum[gid] + rank // 128
                ft = np.asarray(frag_tiles[dt][f], dtype=np.int64)
                gs_ = ft[j] * 128 + rank % 128
                gidx[gs_] = fr['col']
                glrow[gs_] = fr['d'] - gid * GRP
                gval[gs_] = fr['val']
        idx_img = np.zeros((P, C_total), dtype=np.int16)
        for (src, sl, icol0, nipc, ti0, ntl) in calls:
            vec = gidx[ti0 * 128:(ti0 + ntl) * 128].astype(np.int16)
            blk = vec.reshape(-1, 16).T
            idx_img[:, icol0:icol0 + nipc // 16] = np.tile(blk, (8, 1))
        oh = np.zeros((n_tiles, P, GRP), dtype=np.float32)
        ts = np.arange(n_tiles * 128)
        oh[ts // 128, ts % 128, glrow] = gval
        oh_imgs.append(np.ascontiguousarray(
            oh.transpose(1, 0, 2).reshape(P, n_tiles * GRP)).astype(BF16))
        idx_imgs.append(idx_img)
        slot_dbg.append((gidx, glrow, gval))

    return dict(calls=calls, tiles=tiles, batches=batches, C_total=C_total,
                n_tiles=n_tiles, idx_imgs=idx_imgs, oh_imgs=oh_imgs,
                slot_dbg=slot_dbg)


# ---------------------------------------------------------------- builders

def _cls(dt, sl, src):
    """Layer-2 accumulator class of a tile (which W2 applies afterwards)."""
    if dt == 0:
        return 0 if src in (0, 2) else 1      # e00 -> 0, e01 -> 1
    return 0 if src in (0, 2) else sl         # e10 -> 0, e11_k -> k+1


def _w2_slice(dt, k):
    # W2cat cols: [W2_00, W2_01, W2_10, W2_11_0, W2_11_1, W2_11_2]
    return k if dt == 0 else (2 if k == 0 else 2 + k)


def _scatter_tc(nc, tc, sched, layer, srcs, oh_d, idx_d, outs):
    """srcs: {(src_id, sl): (dram 256B-row tensor, 64-col half index)}."""
    calls, tiles, batches = sched['calls'], sched['tiles'], sched['batches']
    psum_bufs = 4 if layer == 1 else 1
    with tc.tile_pool(name="gt", bufs=6) as gtp, \
         tc.tile_pool(name="idxp", bufs=3) as idxp, \
         tc.tile_pool(name="ohp", bufs=2) as ohp, \
         tc.tile_pool(name="ps", bufs=psum_bufs, space="PSUM") as psp, \
         tc.tile_pool(name="ps2", bufs=2, space="PSUM") as psp2, \
         tc.tile_pool(name="stg", bufs=3) as stgp:
        qload = [0.0] * NQ
        for (dt, gs, c0, ncalls, t0, ntl_b) in batches:
            ngs = len(gs)
            ic0 = calls[c0][2]
            last_c = calls[c0 + ncalls - 1]
            ic1 = last_c[2] + last_c[3] // 16
            it = idxp.tile([P, ic1 - ic0], I16, tag="idx")
            nc.sync.dma_start(out=it[:], in_=idx_d[:, ic0:ic1])
            oht = ohp.tile([P, ntl_b * GRP], BF, tag="oh")
            nc.sync.dma_start(
                out=oht[:], in_=oh_d[:, t0 * GRP:(t0 + ntl_b) * GRP])
            ncls = 1 if layer == 1 else (2 if dt == 0 else 4)
            pss = [psp.tile([GRP, ngs * GRP], FP32, tag=f"s{k}", name=f"s{k}")
                   for k in range(ncls)]
            for t_ in pss:
                nc.vector.memset(t_[:], 0.0)
            started = set()
            remaining = {}
            for ti in range(t0, t0 + ntl_b):
                (_ci, _tl, sl, dt_, g, src) = tiles[ti]
                key = (0 if layer == 1 else _cls(dt_, sl, src), g)
                remaining[key] = remaining.get(key, 0) + 1
            for ci in range(c0, c0 + ncalls):
                (src, sl_c, icol0, nipc, ti0, ntl) = calls[ci]
                dram, half = srcs[(src, sl_c)]
                gt = gtp.tile([P, NIPC // 128, P], BF, tag="g256")
                qn = min(range(NQ), key=lambda q: qload[q])
                qload[qn] += 994.0 + nipc * 5.4
                nc.gpsimd.dma_gather(
                    out_ap=gt[:, :ntl, :], in_ap=dram[:, :],
                    idxs_ap=it[:, icol0 - ic0:icol0 - ic0 + nipc // 16],
                    num_idxs=nipc, num_idxs_reg=nipc, elem_size=P,
                    queue_num=qn, single_packet=False)
                for ti in range(ti0, ti0 + ntl):
                    (_ci2, tl, sl, dt_, g, _s) = tiles[ti]
                    lhs = gt[:, tl, half * 64:(half + 1) * 64]
                    ohs = oht[:, (ti - t0) * GRP:(ti - t0 + 1) * GRP]
                    cls = 0 if layer == 1 else _cls(dt_, sl, src)
                    key = (cls, g)
                    remaining[key] -= 1
                    gi = gs.index(g)
                    tgt = pss[cls][:, gi * GRP:(gi + 1) * GRP]
                    started.add(key)
                    if layer == 1:
                        nc.tensor.matmul(tgt, lhsT=ohs, rhs=lhs,
                                         start=False,
                                         stop=remaining[key] == 0,
                                         skip_group_check=True)
                    else:
                        nc.tensor.matmul(tgt, lhsT=lhs, rhs=ohs,
                                         start=False,
                                         stop=remaining[key] == 0,
                                         skip_group_check=True)
            if layer == 1:
                h_dram = outs[dt]
                hp = stgp.tile([GRP, ngs * P], BF, tag="hstg")
                nc.vector.memset(hp[:], 0.0)
                for i, g in enumerate(gs):
                    nc.scalar.activation(
                        out=hp[:, i * P:i * P + GRP],
                        in_=pss[0][:, i * GRP:(i + 1) * GRP],
                        func=mybir.ActivationFunctionType.Relu)
                nc.sync.dma_start(
                    out=h_dram[gs[0] * GRP:gs[0] * GRP + ngs * GRP, :]
                    .rearrange("(g p) f -> p g f", g=ngs),
                    in_=hp[:, :].rearrange("p (g f) -> p g f", g=ngs))
            else:
                z_dram, w2 = outs[dt], outs[2]
                psz = psp2.tile([H2, ngs * GRP], FP32, tag="z")
                nc.vector.memset(psz[:], 0.0)
                for k in range(ncls):
                    sb = stgp.tile([GRP, ngs * GRP], BF, tag=f"sb{k}")
                    nc.vector.tensor_copy(out=sb[:], in_=pss[k][:])
                    wsl = _w2_slice(dt, k)
                    for i in range(ngs):
                        nc.tensor.matmul(
                            psz[:, i * GRP:(i + 1) * GRP],
                            lhsT=w2[:, wsl * H2:(wsl + 1) * H2],
                            rhs=sb[:, i * GRP:(i + 1) * GRP],
                            start=False, stop=(k == ncls - 1),
                            skip_group_check=True)
                zs = stgp.tile([H2, ngs * GRP], FP32, tag="zstg")
                nc.vector.tensor_copy(out=zs[:], in_=psz[:])
                nc.sync.dma_start(
                    out=z_dram[:, gs[0] * GRP:gs[0] * GRP + ngs * GRP],
                    in_=zs[:])


def build_A(sched):
    nc = bacc.Bacc("TRN2", num_swdge_queues=NQ,
                   dynamic_dma_scratch_size=NIPC * 16)
    f0T = nc.dram_tensor("f0T", [P, N0], BF, kind="ExternalInput")
    f1T = nc.dram_tensor("f1T", [P, N1], BF, kind="ExternalInput")
    w1p = nc.dram_tensor("w1p", [F, 2 * H1], BF, kind="ExternalInput")
    w1q = nc.dram_tensor("w1q", [F, 4 * H1], BF, kind="ExternalInput")
    idx_d = nc.dram_tensor("idx", [P, sched['C_total']], I16,
                           kind="ExternalInput")
    oh_d = nc.dram_tensor("oh", [P, sched['n_tiles'] * GRP], BF,
                          kind="ExternalInput")
    h0 = nc.dram_tensor("h0", [PAD0, P], BF, kind="ExternalOutput")
    h1 = nc.dram_tensor("h1", [PAD1, P], BF, kind="ExternalOutput")
    # 256B-row gather sources, two 64-col slices each:
    # x0*: [W1_00 | W1_10] proj of feat0;  x1p1: [W1_01 | W1_11_0],
    # x1p2: [W1_11_1 | W1_11_2] proj of feat1.
    x0a = nc.dram_tensor("x0a", [BANK, 2 * H1], BF)
    x0b = nc.dram_tensor("x0b", [X0B_ROWS, 2 * H1], BF)
    x1p1 = nc.dram_tensor("x1p1", [X1ROWS, 2 * H1], BF)
    x1p2 = nc.dram_tensor("x1p2", [X1ROWS, 2 * H1], BF)

    with tile.TileContext(nc) as tc:
        with tc.tile_pool(name="w", bufs=1) as wp, \
             tc.tile_pool(name="ft", bufs=2) as ftp, \
             tc.tile_pool(name="pp", bufs=4, space="PSUM") as ppp, \
             tc.tile_pool(name="xs", bufs=2) as xsp:
            w1ps = wp.tile([F, 2 * H1], BF)
            w1qs = wp.tile([F, 4 * H1], BF)
            nc.sync.dma_start(out=w1ps[:], in_=w1p[:])
            nc.sync.dma_start(out=w1qs[:], in_=w1q[:])
            # pairs[i] = (bankA, bankB-or-None) for 128-col pair i of proj
            for (fT, n_rows, nchunk, wsb, wcols, pairs) in (
                    (f0T, N0, NCH0, w1ps, 2 * H1, ((x0a, x0b),)),
                    (f1T, N1, NCH1, w1qs, 4 * H1,
                     ((x1p1, None), (x1p2, None)))):
                npair = wcols // P
                for s0 in range(0, nchunk, CH):
                    s1 = min(s0 + CH, nchunk)
                    r0, r1 = s0 * P, min(s1 * P, n_rows)
                    nsc = s1 - s0
                    ft = ftp.tile([P, CH * P], BF, tag="ft")
                    nc.sync.dma_start(out=ft[:, :r1 - r0], in_=fT[:, r0:r1])
                    xs = [xsp.tile([P, CH * P], BF, tag=f"xs{i}",
                                   name=f"xs{i}")
                          for i in range(npair)]
                    for cc in range(s0, s1):
                        pt = ppp.tile([P, wcols], FP32, tag="pp")
                        nc.tensor.matmul(
                            pt[:],
                            lhsT=ft[:, (cc - s0) * P:(cc - s0 + 1) * P],
                            rhs=wsb[:], start=True, stop=True)
                        for i in range(npair):
                            nc.vector.tensor_copy(
                                out=xs[i][:, (cc - s0) * P:(cc - s0 + 1) * P],
                                in_=pt[:, i * P:(i + 1) * P])
                    rp0, rp1 = s0 * P, s1 * P   # padded row range
                    for i in range(npair):
                        xa, xb = pairs[i]
                        if xb is None or rp1 <= BANK:
                            out_ap = xa[rp0:rp1, :]
                        else:
                            out_ap = xb[rp0 - BANK:rp1 - BANK, :]
                        nc.sync.dma_start(
                            out=out_ap.rearrange("(s p) f -> p s f", p=P),
                            in_=xs[i][:, :nsc * P]
                            .rearrange("p (s f) -> p s f", s=nsc))
    with tile.TileContext(nc) as tc:
        srcs = {(0, 0): (x0a, 0), (2, 0): (x0b, 0), (1, 0): (x1p1, 0),
                (0, 1): (x0a, 1), (2, 1): (x0b, 1),
                (1, 1): (x1p1, 1), (1, 2): (x1p2, 0), (1, 3): (x1p2, 1)}
        _scatter_tc(nc, tc, sched, 1, srcs, oh_d, idx_d, (h0, h1))
    nc.finalize()
    return nc


def build_B(sched):
    nc = bacc.Bacc("TRN2", num_swdge_queues=NQ,
                   dynamic_dma_scratch_size=NIPC * 16)
    h0a = nc.dram_tensor("h0a", [BANK, P], BF, kind="ExternalInput")
    h0b = nc.dram_tensor("h0b", [N0 - BANK, P], BF, kind="ExternalInput")
    h1f = nc.dram_tensor("h1f", [N1, P], BF, kind="ExternalInput")
    w2 = nc.dram_tensor("w2", [H1, 6 * H2], BF, kind="ExternalInput")
    idx_d = nc.dram_tensor("idx", [P, sched['C_total']], I16,
                           kind="ExternalInput")
    oh_d = nc.dram_tensor("oh", [P, sched['n_tiles'] * GRP], BF,
                          kind="ExternalInput")
    z0 = nc.dram_tensor("z0", [H2, PAD0], FP32, kind="ExternalOutput")
    z1 = nc.dram_tensor("z1", [H2, PAD1], FP32, kind="ExternalOutput")
    with tile.TileContext(nc) as tc:
        with tc.tile_pool(name="w2p", bufs=1) as w2p:
            w2s = w2p.tile([H1, 6 * H2], BF)
            nc.sync.dma_start(out=w2s[:], in_=w2[:])
            srcs = {(0, 0): (h0a, 0), (2, 0): (h0b, 0), (1, 0): (h1f, 0),
                    (0, 1): (h0a, 0), (2, 1): (h0b, 0),
                    (1, 1): (h1f, 0), (1, 2): (h1f, 0), (1, 3): (h1f, 0)}
            _scatter_tc(nc, tc, sched, 2, srcs, oh_d, idx_d, (z0, z1, w2s))
    nc.finalize()
    return nc


# ---------------------------------------------------------------- kernel

def _trace_available():
    """trace=True needs antenv.axon_hooks (absent in some containers)."""
    try:
        import antenv.axon_hooks  # noqa: F401
        return True
    except Exception:
        return False


def _run(nc, in_maps):
    trace = _trace_available()
    try:
        res = run_bass_kernel_spmd(nc, in_maps, core_ids=list(range(NCORES)),
                                   trace=trace)
        return res, res.exec_time_ns
    except Exception:
        if not trace:
            raise
        res = run_bass_kernel_spmd(nc, in_maps, core_ids=list(range(NCORES)),
                                   trace=False)
        return res, None


def kernel(**inputs):
    e = {k: np.asarray(v) for k, v in inputs.items()}
    sched = _prep(e)

    f0T = np.ascontiguousarray(e['feat0'].T).astype(BF16)
    f1T = np.ascontiguousarray(e['feat1'].T).astype(BF16)
    w1p = np.concatenate([e['W1_00'], e['W1_10']], axis=1).astype(BF16)
    w1q = np.concatenate([e['W1_01'], e['W1_11'][0], e['W1_11'][1],
                          e['W1_11'][2]], axis=1).astype(BF16)
    w2 = np.concatenate([e['W2_00'], e['W2_01'], e['W2_10'], e['W2_11'][0],
                         e['W2_11'][1], e['W2_11'][2]], axis=1).astype(BF16)

    ncA = build_A(sched)
    in_maps = [dict(f0T=f0T, f1T=f1T, w1p=w1p, w1q=w1q,
                    idx=sched['idx_imgs'][c], oh=sched['oh_imgs'][c])
               for c in range(NCORES)]
    resA, HW_TIMES['A'] = _run(ncA, in_maps)

    h0 = np.concatenate([np.asarray(resA.results[c]['h0'])[:D0]
                         for c in range(NCORES)])
    h1 = np.concatenate([np.asarray(resA.results[c]['h1'])[:D1]
                         for c in range(NCORES)])

    ncB = build_B(sched)
    in_mapsB = [dict(h0a=np.ascontiguousarray(h0[:BANK]),
                     h0b=np.ascontiguousarray(h0[BANK:]),
                     h1f=h1, w2=w2,
                     idx=sched['idx_imgs'][c], oh=sched['oh_imgs'][c])
                for c in range(NCORES)]
    resB, HW_TIMES['B'] = _run(ncB, in_mapsB)

    z0 = np.concatenate([np.asarray(resB.results[c]['z0'])[:, :D0].T
                         for c in range(NCORES)]).astype(np.float32)
    z1 = np.concatenate([np.asarray(resB.results[c]['z1'])[:, :D1].T
                         for c in range(NCORES)]).astype(np.float32)
    return np.concatenate([z0, z1], axis=0)


# revision 9
# speedup vs baseline: 1.2038x; 1.2038x over previous
"""Decagon GNN (2-layer multi-relation GCN) on 8 TRN2 NeuronCores.

Destination-node sharding across 8 cores. Edges routed by destination
partition, sorted by destination row, packed into 128-edge tiles. Per tile:
indirect DMA gather (dma_gather) of projected source rows + a TensorE one-hot
matmul performing the val-weighted segment-sum in PSUM.

dma_gather transfers must be multiples of 256B, so projection slices are
packed in pairs: each source array holds two 64-col bf16 slices (256B rows)
and each fragment's matmul consumes its 64-col half. This keeps every
gathered row at the 256B minimum (the 4-queue SWDGE descriptor pipeline is
the bottleneck; its time scales with bytes per descriptor).

Launch A: projections X = feat @ W1 (project-first), layer-1 scatter -> relu
h shards (row-major bf16, padded to 128 cols).
Launch B: host all-gathers h shards, layer-2 scatter -> transposed segment
sums S.T, dense W2 matmuls -> z.T shards. Host assembles [70000, 32] f32.
"""
import sys
import numpy as np

sys.path.insert(0, "/opt/trn_rl_repo")

import ml_dtypes
import concourse.bacc as bacc
import concourse.mybir as mybir
import concourse.tile as tile
from concourse.bass_utils import run_bass_kernel_spmd

BF16 = ml_dtypes.bfloat16
P = 128
NCORES = 8
N0, N1, F, H1, H2, K11 = 50000, 20000, 128, 64, 32, 3
D0, D1 = N0 // NCORES, N1 // NCORES          # 6250, 2500
GRP = 64
G0 = -(-D0 // GRP)                            # 98
G1 = -(-D1 // GRP)                            # 40
PAD0, PAD1 = G0 * GRP, G1 * GRP               # 6272, 2560
BANK = 32768
NB = 4                                        # groups per batch
NIPC = 2048                                   # idxs per dma_gather call
NQ = 4                                        # SWDGE queues
CH = 16                                       # 128-row chunks per proj stage
FP32 = mybir.dt.float32
BF = mybir.dt.bfloat16
I16 = mybir.dt.int16

# padded projection-array row counts (full-chunk stores)
NCH0 = -(-N0 // P)                            # 391
NCH1 = -(-N1 // P)                            # 157
X0ROWS = NCH0 * P                             # 50048
X1ROWS = NCH1 * P                             # 20096
X0B_ROWS = X0ROWS - BANK                      # 17280

HW_TIMES = {}


# ---------------------------------------------------------------- host prep

def _prep(e):
    """Route edges, build the static schedule and per-core idx/onehot images.

    Fragment src ids: 0 = bank A of the N0-source array, 2 = bank B,
    1 = the N1-source array. sl = projection slice of the fragment.
    """
    frag_meta = [[], []]
    per_core = [[[], []] for _ in range(NCORES)]

    def add_frag(dt, rows, cols, vals, dper, src, sl, boff=0, mask_extra=None):
        frag_meta[dt].append((src, sl))
        for c in range(NCORES):
            m = (rows // dper) == c
            if mask_extra is not None:
                m = m & mask_extra
            d = (rows[m] - c * dper).astype(np.int64)
            cc = (cols[m] - boff).astype(np.int64)
            vv = vals[m]
            o = np.argsort(d, kind='stable')
            d, cc, vv = d[o], cc[o], vv[o]
            ng = G0 if dt == 0 else G1
            per_core[c][dt].append(dict(
                d=d, col=cc, val=vv,
                cnt=np.bincount(d // GRP, minlength=ng)))

    ba00 = np.asarray(e['e00_col'] < BANK)
    add_frag(0, e['e00_row'], e['e00_col'], e['e00_val'], D0, 0, 0, 0, ba00)
    add_frag(0, e['e00_row'], e['e00_col'], e['e00_val'], D0, 2, 0, BANK, ~ba00)
    add_frag(0, e['e01_row'], e['e01_col'], e['e01_val'], D0, 1, 0)
    ba10 = np.asarray(e['e10_col'] < BANK)
    add_frag(1, e['e10_row'], e['e10_col'], e['e10_val'], D1, 0, 1, 0, ba10)
    add_frag(1, e['e10_row'], e['e10_col'], e['e10_val'], D1, 2, 1, BANK, ~ba10)
    for k in range(K11):
        add_frag(1, e['e11_row'][k], e['e11_col'][k], e['e11_val'][k],
                 D1, 1, k + 1)

    T = []
    for dt in range(2):
        ng = G0 if dt == 0 else G1
        nf = len(frag_meta[dt])
        Tdt = np.zeros((nf, ng), dtype=np.int64)
        for f in range(nf):
            for c in range(NCORES):
                Tdt[f] = np.maximum(
                    Tdt[f], (per_core[c][dt][f]['cnt'] + 127) // 128)
        T.append(Tdt)

    calls = []        # (src_id, sl, icol0, nipc, ti0, ntl)
    tiles = []        # (call_i, tl, l1_slice, dt, g, src)
    batches = []      # (dt, [groups], call0, ncalls, tile0, ntiles)
    frag_tiles = [[[] for _ in range(T[dt].shape[0])] for dt in range(2)]
    icol = 0
    for dt in range(2):
        ng = G0 if dt == 0 else G1
        for b0 in range(0, ng, NB):
            gs = list(range(b0, min(b0 + NB, ng)))
            bc0, bt0 = len(calls), len(tiles)
            for f in range(T[dt].shape[0]):
                src, sl = frag_meta[dt][f]
                tl_list = [(g, t) for g in gs for t in range(T[dt][f][g])]
                pos = 0
                while pos < len(tl_list):
                    ct = tl_list[pos:pos + NIPC // 128]
                    ci = len(calls)
                    calls.append((src, sl, icol, len(ct) * 128,
                                  len(tiles), len(ct)))
                    for tl, (g, t) in enumerate(ct):
                        frag_tiles[dt][f].append(len(tiles))
                        tiles.append((ci, tl, sl, dt, g, src))
                    icol += len(ct) * 8
                    pos += len(ct)
            batches.append((dt, gs, bc0, len(calls) - bc0, bt0,
                            len(tiles) - bt0))
    C_total, n_tiles = icol, len(tiles)

    idx_imgs, oh_imgs, slot_dbg = [], [], []
    for c in range(NCORES):
        gidx = np.zeros(n_tiles * 128, dtype=np.int64)
        glrow = np.zeros(n_tiles * 128, dtype=np.int64)
        gval = np.zeros(n_tiles * 128, dtype=np.float32)
        for dt in range(2):
            for f in range(T[dt].shape[0]):
                fr = per_core[c][dt][f]
                if len(fr['d']) == 0:
                    continue
                Tcum = np.concatenate([[0], np.cumsum(T[dt][f])])
                gstart = np.concatenate([[0], np.cumsum(fr['cnt'])])
                gid = fr['d'] // GRP
                rank = np.arange(len(fr['d'])) - gstart[gid]
                j = Tcum[gid] + rank // 128
                ft = np.asarray(frag_tiles[dt][f], dtype=np.int64)
                gs_ = ft[j] * 128 + rank % 128
                gidx[gs_] = fr['col']
                glrow[gs_] = fr['d'] - gid * GRP
                gval[gs_] = fr['val']
        idx_img = np.zeros((P, C_total), dtype=np.int16)
        for (src, sl, icol0, nipc, ti0, ntl) in calls:
            vec = gidx[ti0 * 128:(ti0 + ntl) * 128].astype(np.int16)
            blk = vec.reshape(-1, 16).T
            idx_img[:, icol0:icol0 + nipc // 16] = np.tile(blk, (8, 1))
        oh = np.zeros((n_tiles, P, GRP), dtype=np.float32)
        ts = np.arange(n_tiles * 128)
        oh[ts // 128, ts % 128, glrow] = gval
        oh_imgs.append(np.ascontiguousarray(
            oh.transpose(1, 0, 2).reshape(P, n_tiles * GRP)).astype(BF16))
        idx_imgs.append(idx_img)
        slot_dbg.append((gidx, glrow, gval))

    return dict(calls=calls, tiles=tiles, batches=batches, C_total=C_total,
                n_tiles=n_tiles, idx_imgs=idx_imgs, oh_imgs=oh_imgs,
                slot_dbg=slot_dbg)


# ---------------------------------------------------------------- builders

def _cls(dt, sl, src):
    """Layer-2 accumulator class of a tile (which W2 applies afterwards)."""
    if dt == 0:
        return 0 if src in (0, 2) else 1      # e00 -> 0, e01 -> 1
    return 0 if src in (0, 2) else sl         # e10 -> 0, e11_k -> k+1


def _w2_slice(dt, k):
    # W2cat cols: [W2_00, W2_01, W2_10, W2_11_0, W2_11_1, W2_11_2]
    return k if dt == 0 else (2 if k == 0 else 2 + k)


def _scatter_tc(nc, tc, sched, layer, srcs, oh_d, idx_d, outs):
    """srcs: {(src_id, sl): (dram 256B-row tensor, 64-col half index)}."""
    calls, tiles, batches = sched['calls'], sched['tiles'], sched['batches']
    psum_bufs = 4 if layer == 1 else 1
    with tc.tile_pool(name="gt", bufs=12) as gtp, \
         tc.tile_pool(name="idxp", bufs=4) as idxp, \
         tc.tile_pool(name="ohp", bufs=3) as ohp, \
         tc.tile_pool(name="ps", bufs=psum_bufs, space="PSUM") as psp, \
         tc.tile_pool(name="ps2", bufs=2, space="PSUM") as psp2, \
         tc.tile_pool(name="stg", bufs=3) as stgp:
        qload = [0.0] * NQ
        for (dt, gs, c0, ncalls, t0, ntl_b) in batches:
            ngs = len(gs)
            ic0 = calls[c0][2]
            last_c = calls[c0 + ncalls - 1]
            ic1 = last_c[2] + last_c[3] // 16
            it = idxp.tile([P, ic1 - ic0], I16, tag="idx")
            nc.sync.dma_start(out=it[:], in_=idx_d[:, ic0:ic1])
            oht = ohp.tile([P, ntl_b * GRP], BF, tag="oh")
            nc.sync.dma_start(
                out=oht[:], in_=oh_d[:, t0 * GRP:(t0 + ntl_b) * GRP])
            ncls = 1 if layer == 1 else (2 if dt == 0 else 4)
            pss = [psp.tile([GRP, ngs * GRP], FP32, tag=f"s{k}", name=f"s{k}")
                   for k in range(ncls)]
            for t_ in pss:
                nc.vector.memset(t_[:], 0.0)
            started = set()
            remaining = {}
            for ti in range(t0, t0 + ntl_b):
                (_ci, _tl, sl, dt_, g, src) = tiles[ti]
                key = (0 if layer == 1 else _cls(dt_, sl, src), g)
                remaining[key] = remaining.get(key, 0) + 1
            for ci in range(c0, c0 + ncalls):
                (src, sl_c, icol0, nipc, ti0, ntl) = calls[ci]
                dram, half = srcs[(src, sl_c)]
                gt = gtp.tile([P, NIPC // 128, P], BF, tag="g256")
                qn = min(range(NQ), key=lambda q: qload[q])
                qload[qn] += 994.0 + nipc * 5.4
                nc.gpsimd.dma_gather(
                    out_ap=gt[:, :ntl, :], in_ap=dram[:, :],
                    idxs_ap=it[:, icol0 - ic0:icol0 - ic0 + nipc // 16],
                    num_idxs=nipc, num_idxs_reg=nipc, elem_size=P,
                    queue_num=qn, single_packet=False)
                for ti in range(ti0, ti0 + ntl):
                    (_ci2, tl, sl, dt_, g, _s) = tiles[ti]
                    lhs = gt[:, tl, half * 64:(half + 1) * 64]
                    ohs = oht[:, (ti - t0) * GRP:(ti - t0 + 1) * GRP]
                    cls = 0 if layer == 1 else _cls(dt_, sl, src)
                    key = (cls, g)
                    remaining[key] -= 1
                    gi = gs.index(g)
                    tgt = pss[cls][:, gi * GRP:(gi + 1) * GRP]
                    started.add(key)
                    if layer == 1:
                        nc.tensor.matmul(tgt, lhsT=ohs, rhs=lhs,
                                         start=False,
                                         stop=remaining[key] == 0,
                                         skip_group_check=True)
                    else:
                        nc.tensor.matmul(tgt, lhsT=lhs, rhs=ohs,
                                         start=False,
                                         stop=remaining[key] == 0,
                                         skip_group_check=True)
            if layer == 1:
                h_dram = outs[dt]
                hp = stgp.tile([GRP, ngs * P], BF, tag="hstg")
                nc.vector.memset(hp[:], 0.0)
                for i, g in enumerate(gs):
                    nc.scalar.activation(
                        out=hp[:, i * P:i * P + GRP],
                        in_=pss[0][:, i * GRP:(i + 1) * GRP],
                        func=mybir.ActivationFunctionType.Relu)
                nc.sync.dma_start(
                    out=h_dram[gs[0] * GRP:gs[0] * GRP + ngs * GRP, :]
                    .rearrange("(g p) f -> p g f", g=ngs),
                    in_=hp[:, :].rearrange("p (g f) -> p g f", g=ngs))
            else:
                z_dram, w2 = outs[dt], outs[2]
                psz = psp2.tile([H2, ngs * GRP], FP32, tag="z")
                nc.vector.memset(psz[:], 0.0)
                for k in range(ncls):
                    sb = stgp.tile([GRP, ngs * GRP], BF, tag=f"sb{k}")
                    nc.vector.tensor_copy(out=sb[:], in_=pss[k][:])
                    wsl = _w2_slice(dt, k)
                    for i in range(ngs):
                        nc.tensor.matmul(
                            psz[:, i * GRP:(i + 1) * GRP],
                            lhsT=w2[:, wsl * H2:(wsl + 1) * H2],
                            rhs=sb[:, i * GRP:(i + 1) * GRP],
                            start=False, stop=(k == ncls - 1),
                            skip_group_check=True)
                zs = stgp.tile([H2, ngs * GRP], FP32, tag="zstg")
                nc.vector.tensor_copy(out=zs[:], in_=psz[:])
                nc.sync.dma_start(
                    out=z_dram[:, gs[0] * GRP:gs[0] * GRP + ngs * GRP],
                    in_=zs[:])


def build_A(sched):
    nc = bacc.Bacc("TRN2", num_swdge_queues=NQ,
                   dynamic_dma_scratch_size=NIPC * 16)
    f0T = nc.dram_tensor("f0T", [P, N0], BF, kind="ExternalInput")
    f1T = nc.dram_tensor("f1T", [P, N1], BF, kind="ExternalInput")
    w1p = nc.dram_tensor("w1p", [F, 2 * H1], BF, kind="ExternalInput")
    w1q = nc.dram_tensor("w1q", [F, 4 * H1], BF, kind="ExternalInput")
    idx_d = nc.dram_tensor("idx", [P, sched['C_total']], I16,
                           kind="ExternalInput")
    oh_d = nc.dram_tensor("oh", [P, sched['n_tiles'] * GRP], BF,
                          kind="ExternalInput")
    h0 = nc.dram_tensor("h0", [PAD0, P], BF, kind="ExternalOutput")
    h1 = nc.dram_tensor("h1", [PAD1, P], BF, kind="ExternalOutput")
    # 256B-row gather sources, two 64-col slices each:
    # x0*: [W1_00 | W1_10] proj of feat0;  x1p1: [W1_01 | W1_11_0],
    # x1p2: [W1_11_1 | W1_11_2] proj of feat1.
    x0a = nc.dram_tensor("x0a", [BANK, 2 * H1], BF)
    x0b = nc.dram_tensor("x0b", [X0B_ROWS, 2 * H1], BF)
    x1p1 = nc.dram_tensor("x1p1", [X1ROWS, 2 * H1], BF)
    x1p2 = nc.dram_tensor("x1p2", [X1ROWS, 2 * H1], BF)

    with tile.TileContext(nc) as tc:
        with tc.tile_pool(name="w", bufs=1) as wp, \
             tc.tile_pool(name="ft", bufs=2) as ftp, \
             tc.tile_pool(name="pp", bufs=4, space="PSUM") as ppp, \
             tc.tile_pool(name="xs", bufs=2) as xsp:
            w1ps = wp.tile([F, 2 * H1], BF)
            w1qs = wp.tile([F, 4 * H1], BF)
            nc.sync.dma_start(out=w1ps[:], in_=w1p[:])
            nc.sync.dma_start(out=w1qs[:], in_=w1q[:])
            # pairs[i] = (bankA, bankB-or-None) for 128-col pair i of proj
            for (fT, n_rows, nchunk, wsb, wcols, pairs) in (
                    (f0T, N0, NCH0, w1ps, 2 * H1, ((x0a, x0b),)),
                    (f1T, N1, NCH1, w1qs, 4 * H1,
                     ((x1p1, None), (x1p2, None)))):
                npair = wcols // P
                for s0 in range(0, nchunk, CH):
                    s1 = min(s0 + CH, nchunk)
                    r0, r1 = s0 * P, min(s1 * P, n_rows)
                    nsc = s1 - s0
                    ft = ftp.tile([P, CH * P], BF, tag="ft")
                    nc.sync.dma_start(out=ft[:, :r1 - r0], in_=fT[:, r0:r1])
                    xs = [xsp.tile([P, CH * P], BF, tag=f"xs{i}",
                                   name=f"xs{i}")
                          for i in range(npair)]
                    for cc in range(s0, s1):
                        pt = ppp.tile([P, wcols], FP32, tag="pp")
                        nc.tensor.matmul(
                            pt[:],
                            lhsT=ft[:, (cc - s0) * P:(cc - s0 + 1) * P],
                            rhs=wsb[:], start=True, stop=True)
                        for i in range(npair):
                            nc.vector.tensor_copy(
                                out=xs[i][:, (cc - s0) * P:(cc - s0 + 1) * P],
                                in_=pt[:, i * P:(i + 1) * P])
                    rp0, rp1 = s0 * P, s1 * P   # padded row range
                    for i in range(npair):
                        xa, xb = pairs[i]
                        if xb is None or rp1 <= BANK:
                            out_ap = xa[rp0:rp1, :]
                        else:
                            out_ap = xb[rp0 - BANK:rp1 - BANK, :]
                        nc.sync.dma_start(
                            out=out_ap.rearrange("(s p) f -> p s f", p=P),
                            in_=xs[i][:, :nsc * P]
                            .rearrange("p (s f) -> p s f", s=nsc))
    with tile.TileContext(nc) as tc:
        srcs = {(0, 0): (x0a, 0), (2, 0): (x0b, 0), (1, 0): (x1p1, 0),
                (0, 1): (x0a, 1), (2, 1): (x0b, 1),
                (1, 1): (x1p1, 1), (1, 2): (x1p2, 0), (1, 3): (x1p2, 1)}
        _scatter_tc(nc, tc, sched, 1, srcs, oh_d, idx_d, (h0, h1))
    nc.finalize()
    return nc


def build_B(sched):
    nc = bacc.Bacc("TRN2", num_swdge_queues=NQ,
                   dynamic_dma_scratch_size=NIPC * 16)
    h0a = nc.dram_tensor("h0a", [BANK, P], BF, kind="ExternalInput")
    h0b = nc.dram_tensor("h0b", [N0 - BANK, P], BF, kind="ExternalInput")
    h1f = nc.dram_tensor("h1f", [N1, P], BF, kind="ExternalInput")
    w2 = nc.dram_tensor("w2", [H1, 6 * H2], BF, kind="ExternalInput")
    idx_d = nc.dram_tensor("idx", [P, sched['C_total']], I16,
                           kind="ExternalInput")
    oh_d = nc.dram_tensor("oh", [P, sched['n_tiles'] * GRP], BF,
                          kind="ExternalInput")
    z0 = nc.dram_tensor("z0", [H2, PAD0], FP32, kind="ExternalOutput")
    z1 = nc.dram_tensor("z1", [H2, PAD1], FP32, kind="ExternalOutput")
    with tile.TileContext(nc) as tc:
        with tc.tile_pool(name="w2p", bufs=1) as w2p:
            w2s = w2p.tile([H1, 6 * H2], BF)
            nc.sync.dma_start(out=w2s[:], in_=w2[:])
            srcs = {(0, 0): (h0a, 0), (2, 0): (h0b, 0), (1, 0): (h1f, 0),
                    (0, 1): (h0a, 0), (2, 1): (h0b, 0),
                    (1, 1): (h1f, 0), (1, 2): (h1f, 0), (1, 3): (h1f, 0)}
            _scatter_tc(nc, tc, sched, 2, srcs, oh_d, idx_d, (z0, z1, w2s))
    nc.finalize()
    return nc


# ---------------------------------------------------------------- kernel

def _trace_available():
    """trace=True needs antenv.axon_hooks (absent in some containers)."""
    try:
        import antenv.axon_hooks  # noqa: F401
        return True
    except Exception:
        return False


def _run(nc, in_maps):
    trace = _trace_available()
    try:
        res = run_bass_kernel_spmd(nc, in_maps, core_ids=list(range(NCORES)),
                                   trace=trace)
        return res, res.exec_time_ns
    except Exception:
        if not trace:
            raise
        res = run_bass_kernel_spmd(nc, in_maps, core_ids=list(range(NCORES)),
                                   trace=False)
        return res, None


def kernel(**inputs):
    e = {k: np.asarray(v) for k, v in inputs.items()}
    sched = _prep(e)

    f0T = np.ascontiguousarray(e['feat0'].T).astype(BF16)
    f1T = np.ascontiguousarray(e['feat1'].T).astype(BF16)
    w1p = np.concatenate([e['W1_00'], e['W1_10']], axis=1).astype(BF16)
    w1q = np.concatenate([e['W1_01'], e['W1_11'][0], e['W1_11'][1],
                          e['W1_11'][2]], axis=1).astype(BF16)
    w2 = np.concatenate([e['W2_00'], e['W2_01'], e['W2_10'], e['W2_11'][0],
                         e['W2_11'][1], e['W2_11'][2]], axis=1).astype(BF16)

    ncA = build_A(sched)
    in_maps = [dict(f0T=f0T, f1T=f1T, w1p=w1p, w1q=w1q,
                    idx=sched['idx_imgs'][c], oh=sched['oh_imgs'][c])
               for c in range(NCORES)]
    resA, HW_TIMES['A'] = _run(ncA, in_maps)

    h0 = np.concatenate([np.asarray(resA.results[c]['h0'])[:D0]
                         for c in range(NCORES)])
    h1 = np.concatenate([np.asarray(resA.results[c]['h1'])[:D1]
                         for c in range(NCORES)])

    ncB = build_B(sched)
    in_mapsB = [dict(h0a=np.ascontiguousarray(h0[:BANK]),
                     h0b=np.ascontiguousarray(h0[BANK:]),
                     h1f=h1, w2=w2,
                     idx=sched['idx_imgs'][c], oh=sched['oh_imgs'][c])
                for c in range(NCORES)]
    resB, HW_TIMES['B'] = _run(ncB, in_mapsB)

    z0 = np.concatenate([np.asarray(resB.results[c]['z0'])[:, :D0].T
                         for c in range(NCORES)]).astype(np.float32)
    z1 = np.concatenate([np.asarray(resB.results[c]['z1'])[:, :D1].T
                         for c in range(NCORES)]).astype(np.float32)
    return np.concatenate([z0, z1], axis=0)
